# revision 2
# baseline (speedup 1.0000x reference)
"""Trainium2 Bass kernel for nn_CrossAttention (B=2, N=2048, D=768, H=12).

Sharding: (batch, head-group) across 8 cores — core c handles batch c//4 and
heads [3g, 3g+2] where g = c%4. Attention is fully local per (batch, head).

All matmuls in bf16 (full-rate PE). Inputs are converted to bf16 on the HOST
(halves input DMA, no on-device conversion). Per-core schedule:
  - x blocks (512-col) DMA'd in dependency order; q/k/v projections consume
    each block immediately (bias added on the PSUM->SBUF copy, bf16 out).
  - attention is split into 2 i-passes of 1024 cols per head so PSUM fits:
    pj 2 banks + pss 2x[128,1024] (4) + po 2x[65,512] (2) = 8 banks.
  - per (head, pass, j-group of 4 jc): S^T tile via PE, exp on ACT (bf16
    out), AV accumulates [65,512] in PSUM; group result flushed (DVE
    copy/add) into an SBUF accumulator [65, N] per head. Row 64 is the
    softmax denominator via a ones-column in v'.
  - divide: DVE reciprocal + gpsimd partition broadcast + DVE mul, DMA out.
ACT (exp, ~99us) and PE (~112us) are co-bottlenecks; schedule keeps both
dense from ~4us after start.
"""

import sys

if "/opt/trn_rl_repo" not in sys.path:
    sys.path.insert(0, "/opt/trn_rl_repo")

import numpy as np
import ml_dtypes

import concourse.bass as bass
import concourse.tile as tile
from concourse import bacc, mybir
from concourse.bass_utils import run_bass_kernel_spmd

F32 = mybir.dt.float32
BF16 = mybir.dt.bfloat16
AF = mybir.ActivationFunctionType
BF = ml_dtypes.bfloat16

B, N, D, H, PD = 2, 2048, 768, 12, 64
HPC = 3   # heads per core
KC = 6    # contraction chunks: 768 / 128
NJ = 16   # j chunks of 128
NB = 4    # 512-col blocks per x tensor
WQK = HPC * PD       # 192
VW = HPC * (PD + 1)  # 195 per j-tile in v' (64 v cols + ones col per head)

# test harness hooks
TRACE = False
LAST_RESULTS = None

_cache: dict = {}


def _emit(tc, xq_t, xkv_t, wq_t, wk_t, wv_t, bq, bk, bv, o_t, loop_iters=1):
    if loop_iters > 1:
        with tc.For_i(0, loop_iters, 1):
            _emit_body(tc, xq_t, xkv_t, wq_t, wk_t, wv_t, bq, bk, bv, o_t)
    else:
        _emit_body(tc, xq_t, xkv_t, wq_t, wk_t, wv_t, bq, bk, bv, o_t)


def _emit_body(tc, xq_t, xkv_t, wq_t, wk_t, wv_t, bq, bk, bv, o_t):
    nc = tc.nc

    import contextlib

    with contextlib.ExitStack() as ctx:
        persist = ctx.enter_context(tc.tile_pool(name="persist", bufs=1))
        xp = ctx.enter_context(tc.tile_pool(name="xp", bufs=3))
        expp = ctx.enter_context(tc.tile_pool(name="expp", bufs=4))
        outp = ctx.enter_context(tc.tile_pool(name="outp", bufs=3))
        smallp = ctx.enter_context(tc.tile_pool(name="smallp", bufs=2))
        pjp = ctx.enter_context(tc.tile_pool(name="pjp", bufs=2, space="PSUM"))
        psp = ctx.enter_context(tc.tile_pool(name="psp", bufs=2, space="PSUM"))
        pop = ctx.enter_context(tc.tile_pool(name="pop", bufs=2, space="PSUM"))

        # ---- biases ----
        bq_sb = persist.tile([128, 2], F32)
        bk_sb = persist.tile([128, 2], F32)
        nc.sync.dma_start(bq_sb[:, 0:1], bq[0:128, :])
        nc.sync.dma_start(bq_sb[0:64, 1:2], bq[128:192, :])
        nc.sync.dma_start(bk_sb[:, 0:1], bk[0:128, :])
        nc.sync.dma_start(bk_sb[0:64, 1:2], bk[128:192, :])
        bv_sb = persist.tile([1, WQK], BF16)
        nc.sync.dma_start(bv_sb[:], bv[:])

        # ones row [1,128] bf16 for the v-bias rank-1 matmul
        ones_f = persist.tile([1, 128], F32)
        nc.vector.memset(ones_f[:], 1.0)
        ones_bf = persist.tile([1, 128], BF16)
        nc.vector.tensor_copy(ones_bf[:], ones_f[:])

        # v' and its ones columns (column 64 of each head block)
        v_sb = persist.tile([128, NJ * VW], BF16)
        ones48 = persist.tile([128, HPC * NJ], F32)
        nc.vector.memset(ones48[:], 1.0)
        dst_ones = v_sb[:].rearrange("p (g c) -> p g c", c=PD + 1)[:, :, PD : PD + 1]
        nc.vector.tensor_copy(dst_ones, ones48[:].rearrange("p (g o) -> p g o", o=1))

        # ---- weights: bf16 direct DMA ----
        def load_w(wdram):
            w_sb = persist.tile([128, KC * WQK], BF16, name=wdram.name + "_sb")
            nc.sync.dma_start(
                w_sb[:].rearrange("p (kc m) -> p kc m", kc=KC),
                wdram.rearrange("(kc p) m -> p kc m", p=128),
            )
            return w_sb

        wq_sb = load_w(wq_t)
        wk_sb = load_w(wk_t)
        wv_sb = load_w(wv_t)

        qT01 = persist.tile([128, N], BF16)
        qT2 = persist.tile([64, N], BF16)
        kT01 = persist.tile([128, N], BF16)
        kT2 = persist.tile([64, N], BF16)
        accum = [
            persist.tile([PD + 1, N], F32, name=f"accum{h}") for h in range(HPC)
        ]

        xq_v = xq_t.rearrange("(kc p) i -> p kc i", p=128)
        xkv_v = xkv_t.rearrange("(kc p) i -> p kc i", p=128)

        def qproj(ic):
            xb = xp.tile([128, KC * 512], BF16, tag="xb", name=f"xq{ic}")
            nc.sync.dma_start(
                xb[:].rearrange("p (kc i) -> p kc i", kc=KC),
                xq_v[:, :, ic * 512 : (ic + 1) * 512],
            )
            for grp, m0, msz, dst in ((0, 0, 128, qT01), (1, 128, 64, qT2)):
                ps = pjp.tile([msz, 512], F32, tag="pj", name=f"q{ic}g{grp}")
                for kc in range(KC):
                    nc.tensor.matmul(
                        ps[:],
                        wq_sb[:, kc * WQK + m0 : kc * WQK + m0 + msz],
                        xb[:, kc * 512 : (kc + 1) * 512],
                        start=(kc == 0),
                        stop=(kc == KC - 1),
                    )
                nc.vector.tensor_scalar_add(
                    dst[:, ic * 512 : (ic + 1) * 512], ps[:], bq_sb[0:msz, grp : grp + 1]
                )

        def kvblock(jb):
            xb = xp.tile([128, KC * 512], BF16, tag="xb", name=f"xkv{jb}")
            nc.sync.dma_start(
                xb[:].rearrange("p (kc i) -> p kc i", kc=KC),
                xkv_v[:, :, jb * 512 : (jb + 1) * 512],
            )
            for grp, m0, msz, dst in ((0, 0, 128, kT01), (1, 128, 64, kT2)):
                ps = pjp.tile([msz, 512], F32, tag="pj", name=f"k{jb}g{grp}")
                for kc in range(KC):
                    nc.tensor.matmul(
                        ps[:],
                        wk_sb[:, kc * WQK + m0 : kc * WQK + m0 + msz],
                        xb[:, kc * 512 : (kc + 1) * 512],
                        start=(kc == 0),
                        stop=(kc == KC - 1),
                    )
                nc.vector.tensor_scalar_add(
                    dst[:, jb * 512 : (jb + 1) * 512], ps[:], bk_sb[0:msz, grp : grp + 1]
                )
            for jt in range(4):
                pv = pjp.tile([128, WQK], F32, tag="pj", name=f"v{jb}{jt}")
                for kc in range(KC):
                    nc.tensor.matmul(
                        pv[:],
                        xb[:, kc * 512 + jt * 128 : kc * 512 + (jt + 1) * 128],
                        wv_sb[:, kc * WQK : (kc + 1) * WQK],
                        start=(kc == 0),
                        stop=False,
                    )
                nc.tensor.matmul(pv[:], ones_bf[:], bv_sb[:], start=False, stop=True)
                jt_abs = jb * 4 + jt
                src = pv[:, 0:WQK].rearrange("p (h c) -> p h c", h=HPC)
                dstv = v_sb[:, jt_abs * VW : (jt_abs + 1) * VW].rearrange(
                    "p (h c) -> p h c", h=HPC
                )[:, :, 0:PD]
                nc.vector.tensor_copy(dstv, src)

        def att_group(h, p, g):
            qT_h = qT01[h * 64 : (h + 1) * 64, :] if h < 2 else qT2[:]
            kT_h = kT01[h * 64 : (h + 1) * 64, :] if h < 2 else kT2[:]
            pos = [
                pop.tile([PD + 1, 512], F32, tag="po", name=f"po{h}{p}{g}{q}")
                for q in range(2)
            ]
            for jc in range(g * 4, g * 4 + 4):
                pss = psp.tile([128, 1024], F32, tag="ps", name=f"ps{h}{p}{jc}")
                for q in range(2):
                    nc.tensor.matmul(
                        pss[:, q * 512 : (q + 1) * 512],
                        kT_h[:, jc * 128 : (jc + 1) * 128],
                        qT_h[:, p * 1024 + q * 512 : p * 1024 + (q + 1) * 512],
                        start=True,
                        stop=True,
                    )
                ex = expp.tile([128, 1024], BF16, tag="ex", name=f"ex{h}{p}{jc}")
                nc.scalar.activation(ex[:], pss[:], AF.Exp)
                vp = v_sb[:, jc * VW + h * (PD + 1) : jc * VW + (h + 1) * (PD + 1)]
                for q in range(2):
                    nc.tensor.matmul(
                        pos[q][:],
                        vp,
                        ex[:, q * 512 : (q + 1) * 512],
                        start=(jc == g * 4),
                        stop=(jc == g * 4 + 3),
                    )
            for q in range(2):
                sl = accum[h][:, p * 1024 + q * 512 : p * 1024 + (q + 1) * 512]
                if g == 0:
                    nc.vector.tensor_copy(sl, pos[q][:])
                else:
                    nc.vector.tensor_add(sl, sl, pos[q][:])

        def div_out(h, p):
            for q in range(2):
                col = p * 1024 + q * 512
                recip = smallp.tile([1, 512], F32, tag="rc", name=f"rc{h}{p}{q}")
                nc.vector.reciprocal(recip[:], accum[h][PD : PD + 1, col : col + 512])
                bcast = smallp.tile([64, 512], F32, tag="bc", name=f"bc{h}{p}{q}")
                nc.gpsimd.partition_broadcast(bcast[:], recip[:])
                out_sb = outp.tile([64, 512], F32, tag="out", name=f"o{h}{p}{q}")
                nc.vector.tensor_mul(out_sb[:], accum[h][0:PD, col : col + 512], bcast[:])
                nc.sync.dma_start(o_t[h, :, col : col + 512], out_sb[:])

        # ---- schedule: feed ACT asap, keep PE dense ----
        kvblock(0)
        qproj(0)
        qproj(1)
        for h in range(HPC):
            att_group(h, 0, 0)
        kvblock(1)
        for h in range(HPC):
            att_group(h, 0, 1)
        kvblock(2)
        for h in range(HPC):
            att_group(h, 0, 2)
        kvblock(3)
        qproj(2)
        qproj(3)
        for h in range(HPC):
            att_group(h, 0, 3)
        for h in range(HPC):
            div_out(h, 0)
        for h in range(HPC):
            for g in range(4):
                att_group(h, 1, g)
            div_out(h, 1)


def _build(loop_iters=1):
    key = ("nc", loop_iters)
    if key in _cache:
        return _cache[key]
    nc = bacc.Bacc("TRN2", target_bir_lowering=False, debug=False, num_devices=8)
    xq_t = nc.dram_tensor("xq_t", [D, N], BF16, kind="ExternalInput").ap()
    xkv_t = nc.dram_tensor("xkv_t", [D, N], BF16, kind="ExternalInput").ap()
    wq_t = nc.dram_tensor("wq_t", [D, WQK], BF16, kind="ExternalInput").ap()
    wk_t = nc.dram_tensor("wk_t", [D, WQK], BF16, kind="ExternalInput").ap()
    wv_t = nc.dram_tensor("wv_t", [D, WQK], BF16, kind="ExternalInput").ap()
    bq = nc.dram_tensor("bq", [WQK, 1], F32, kind="ExternalInput").ap()
    bk = nc.dram_tensor("bk", [WQK, 1], F32, kind="ExternalInput").ap()
    bv = nc.dram_tensor("bv", [1, WQK], BF16, kind="ExternalInput").ap()
    o_t = nc.dram_tensor("o_t", [HPC, PD, N], F32, kind="ExternalOutput").ap()
    with tile.TileContext(nc) as tc:
        _emit(tc, xq_t, xkv_t, wq_t, wk_t, wv_t, bq, bk, bv, o_t, loop_iters)
    nc.compile()
    _cache[key] = nc
    return nc


def _shard(x1, x2, Wq, bq, Wkv, bkv):
    in_maps = []
    for c in range(8):
        b, g = divmod(c, 4)
        hd = slice(192 * g, 192 * (g + 1))
        vd = slice(D + hd.start, D + hd.stop)
        in_maps.append(
            {
                "xq_t": np.ascontiguousarray(x2[b].T).astype(BF),
                "xkv_t": np.ascontiguousarray(x1[b].T).astype(BF),
                "wq_t": np.ascontiguousarray(Wq[hd].T).astype(BF),
                "wk_t": np.ascontiguousarray(Wkv[hd].T).astype(BF),
                "wv_t": np.ascontiguousarray(Wkv[vd].T).astype(BF),
                "bq": np.ascontiguousarray(bq[hd].reshape(-1, 1)),
                "bk": np.ascontiguousarray(bkv[hd].reshape(-1, 1)),
                "bv": np.ascontiguousarray(bkv[vd].reshape(1, -1)).astype(BF),
            }
        )
    return in_maps


def kernel(x1, x2, Wq, bq, Wkv, bkv):
    global LAST_RESULTS
    x1 = np.asarray(x1, dtype=np.float32)
    x2 = np.asarray(x2, dtype=np.float32)
    Wq = np.asarray(Wq, dtype=np.float32)
    bq = np.asarray(bq, dtype=np.float32)
    Wkv = np.asarray(Wkv, dtype=np.float32)
    bkv = np.asarray(bkv, dtype=np.float32)

    nc = _build()
    in_maps = _shard(x1, x2, Wq, bq, Wkv, bkv)
    res = run_bass_kernel_spmd(nc, in_maps, core_ids=list(range(8)), trace=TRACE)
    LAST_RESULTS = res

    out = np.empty((B, H, N, PD), np.float32)
    for c in range(8):
        b, g = divmod(c, 4)
        ot = res.results[c]["o_t"]  # (3, 64, 2048)
        out[b, 3 * g : 3 * g + 3] = ot.transpose(0, 2, 1)
    return out.reshape(B, N, D)
